# revision 18
# baseline (speedup 1.0000x reference)
"""BCH/RS systematic encoder kernel for Trainium2 (8 NeuronCores, data parallel).

Computes out = concat([msg, (msg @ Gp) mod 2], axis=-1) for
msg [16384, 1000] f32 of 0/1 bits and Gp [1000, 256] f32 of 0/1 bits.

v6 design (per core, 2048 rows, 8 superchunks of 2x128 rows). Earlier
xbar-transpose variants all serialized: the Tile scheduler cross-
serializes HWDGE DMAs against in-flight DMA_TRANSPOSEs (deadlock guard),
and the xbar's thousands of tiny packets starve any concurrent bulk
stream at the SDMA packet round-robin. So: NO DMA transposes at all.

  - HWDGE f32 loads (sync ring, all emitted upfront, in-order drain)
  - HWDGE f32 msg-stores (scalar ring) a -> out[:, :1000]; pure stream,
    never waits on compute
  - DVE cast a -> abf bf16 (0/1 exact), pad cols 1000:1024 memset
  - PE transposes (matmul transpose-mode vs identity): 16 tiles
    [128m,128k] -> PSUM [128k,128m] per superchunk, in 4 groups of 4
  - ACT evicts each PSUM group -> b [k, m] bf16 SBUF (also ACT's only job)
  - 16 accumulating bf16 matmuls: acc[m, 2*256] += b_k.T @ Gp_k
  - PE stream interleaves T(it+1) with mm(it) so the Tensor engine never
    idles: TRN2 PE p-states are 0.65/1.2/2.4 GHz and only reach 2.4 GHz
    after ~3us of CONTINUOUS execution (gapped bursts run at 1.2 GHz)
  - parity mod-2 on DVE: CAST psum f32->i32, AND 1, CAST i32->f32
  - HWDGE parity stores (sync ring, after all loads)

HBM traffic/core = 8.19 MB read + 10.29 MB write (the minimum); measured
single-direction stream rate ~400 GB/s/core -> ~46 us stream floor.
"""

import os
import sys

import numpy as np

if os.path.isdir("/opt/trn_rl_repo") and "/opt/trn_rl_repo" not in sys.path:
    sys.path.insert(0, "/opt/trn_rl_repo")

import ml_dtypes

import concourse.bacc as bacc
import concourse.mybir as mybir
import concourse.tile as tile
from concourse.bass_utils import run_bass_kernel_spmd

BATCH = 16384
MSG = 1000
NPAR = 256
NCORES = 8
ROWS = BATCH // NCORES  # 2048
P = 128
KCH = 8  # k chunks; padded K = 1024
KPAD = KCH * P
SC = 2  # m-chunks per superchunk
NG = SC * KCH // 4  # transpose groups of 4 tiles per superchunk

# test.py pokes these for profiling
TRACE = False
LAST_RESULT = None

_CACHE = {}


def build_nc(rows=ROWS):
    """Emit the Bass/Tile IR for one core handling `rows` rows."""
    n_it = rows // (SC * P)
    nc = bacc.Bacc("TRN2", target_bir_lowering=False, debug=False)
    msg = nc.dram_tensor("msg", [rows, MSG], mybir.dt.float32, kind="ExternalInput")
    gp = nc.dram_tensor("gp", [P, KCH * NPAR], mybir.dt.bfloat16, kind="ExternalInput")
    ident = nc.dram_tensor("ident", [P, P], mybir.dt.float32, kind="ExternalInput")
    out = nc.dram_tensor(
        "out", [rows, MSG + NPAR], mybir.dt.float32, kind="ExternalOutput"
    )

    msg3 = msg[:, :].rearrange("(s c p) k -> s c p k", c=SC, p=P)
    out3 = out[:, :].rearrange("(s c p) k -> s c p k", c=SC, p=P)

    with tile.TileContext(nc) as tc:
        with (
            # every buffer's full live range fits in its pool, so the Tile
            # scheduler never emits slot-release relay waits (those ride
            # engine queues and serialize the pipeline cross-superchunk)
            tc.tile_pool(name="gpool", bufs=1) as gpool,
            tc.tile_pool(name="apool", bufs=n_it // 2 + 2) as apool,
            tc.tile_pool(name="bpool", bufs=6) as bpool,
            tc.tile_pool(name="parpool", bufs=n_it) as parpool,
            tc.tile_pool(name="ipool", bufs=4) as ipool,
            tc.tile_pool(name="tpool", bufs=6, space="PSUM") as tpool,
            tc.tile_pool(name="ppool", bufs=2, space="PSUM") as ppool,
        ):
            # constants on the SWDGE queue: the HWDGE lane pool (8 sems,
            # round-robin, re-armed on reuse) then carries ONLY 8 loads +
            # 8 msg-stores -> loads take the first 8 lanes and never wait,
            # stores arm on long-completed loads
            idsb = gpool.tile([P, P], mybir.dt.float32)
            nc.gpsimd.dma_start(out=idsb[:, :], in_=ident[:, :])
            gsb = gpool.tile([P, KCH * NPAR], mybir.dt.bfloat16)
            nc.gpsimd.dma_start(out=gsb[:, :], in_=gp[:, :])

            a_t = {}
            b_t = {}
            acc_t = {}

            def emit_load(it):
                # f32 tile with pad cols 1000:1024 (memset once; the PE
                # transposes read the pad, matmuls contract only k<1000)
                a = apool.tile([P, SC, KPAD], mybir.dt.float32, tag="a")
                nc.sync.dma_start(
                    out=a[:, :, 0:MSG],
                    in_=msg3[it, :, :, :].rearrange("c p k -> p c k"),
                )
                a_t[it] = a

            def emit_msg_store(it):
                # model-time offset: keeps the scheduler's global tick order
                # (which drives HWDGE sem-lane assignment) from interleaving
                # store lanes among the load lanes -> loads never arm-wait
                # on store completions
                with tc.tile_wait_until(0.030 + 0.002 * it):
                    nc.scalar.dma_start(
                        out=out3[it, :, :, 0:MSG].rearrange("c p k -> p c k"),
                        in_=a_t[it][:, :, 0:MSG],
                    )


            def emit_trans(it):
                # PE transpose-mode straight from the f32 a-tile in 4-tile
                # groups -> PSUM f32; evictions (f32 -> bf16 lhsT layout
                # b[q, c*KCH+kb, m]) split across ACT and DVE so neither
                # becomes the serial hot engine
                a = a_t[it]
                b = bpool.tile([P, SC * KCH, P], mybir.dt.bfloat16, tag="b")
                for g in range(NG):
                    t = tpool.tile([P, 4, P], mybir.dt.float32, tag="t")
                    for j in range(4):
                        c, kb = divmod(g * 4 + j, KCH)
                        nc.tensor.transpose(
                            t[:, j, :],
                            a[:, c, kb * P : (kb + 1) * P],
                            idsb[:, :],
                        )
                    if g % 2 == 0:
                        nc.scalar.copy(b[:, g * 4 : (g + 1) * 4, :], t[:, :, :])
                    else:
                        nc.vector.tensor_copy(
                            b[:, g * 4 : (g + 1) * 4, :], t[:, :, :]
                        )
                b_t[it] = b

            def emit_mm(it):
                b = b_t.pop(it)
                acc = ppool.tile([P, SC * NPAR], mybir.dt.float32, tag="acc")
                for c in range(SC):
                    for kb in range(KCH):
                        kk = P if kb < KCH - 1 else MSG - (KCH - 1) * P  # 104 tail
                        nc.tensor.matmul(
                            acc[:, c * NPAR : (c + 1) * NPAR],
                            b[0:kk, c * KCH + kb, :],
                            gsb[0:kk, kb * NPAR : (kb + 1) * NPAR],
                            start=(kb == 0),
                            stop=(kb == KCH - 1),
                        )
                acc_t[it] = acc

            def emit_parity(it):
                # mod 2 on DVE only: psum f32 -> i32 (numeric cast, exact
                # for integer sums <= 1000), AND 1, i32 -> f32
                ci = ipool.tile([P, SC * NPAR], mybir.dt.int32, tag="ci")
                nc.scalar.copy(ci[:, :], acc_t.pop(it)[:, :])
                nc.vector.tensor_scalar(
                    ci[:, :], ci[:, :], 1, None, mybir.AluOpType.bitwise_and
                )
                par = parpool.tile([P, SC, NPAR], mybir.dt.float32, tag="par")
                nc.vector.tensor_copy(
                    par[:, :, :].rearrange("p c n -> p (c n)"), ci[:, :]
                )
                return par

            def emit_par_store(it, par):
                # SWDGE: fires right after parity lands instead of queueing
                # behind the load/store streams on the HWDGE rings
                nc.gpsimd.dma_start(
                    out=out3[it, :, :, MSG : MSG + NPAR].rearrange("c p k -> p c k"),
                    in_=par[:, :, :],
                )

            for i in range(n_it):
                emit_load(i)
            for i in range(n_it):
                nc.vector.memset(a_t[i][:, :, MSG:KPAD], 0)
            emit_trans(0)
            for it in range(n_it):
                emit_msg_store(it)
                if it + 1 < n_it:
                    emit_trans(it + 1)
                emit_mm(it)
                par = emit_parity(it)
                emit_par_store(it, par)

    nc.compile()
    return nc


def prep_gp(Gp):
    """Pad Gp to 1024 rows and swizzle to the [128, 8*256] bf16 SBUF layout."""
    gp = np.asarray(Gp, dtype=np.float32)
    gp_pad = np.zeros((KPAD, NPAR), dtype=np.float32)
    gp_pad[:MSG] = gp
    gsw = gp_pad.reshape(KCH, P, NPAR).transpose(1, 0, 2).reshape(P, KCH * NPAR)
    return np.ascontiguousarray(gsw).astype(ml_dtypes.bfloat16)


def prep_ident():
    return np.eye(P, dtype=np.float32)


def kernel(message_bits, Gp):
    global LAST_RESULT
    msg = np.ascontiguousarray(np.asarray(message_bits, dtype=np.float32))
    assert msg.shape == (BATCH, MSG), msg.shape
    gsw = prep_gp(Gp)
    idn = prep_ident()

    if "nc" not in _CACHE:
        _CACHE["nc"] = build_nc()
    nc = _CACHE["nc"]

    in_maps = [
        {"msg": msg[i * ROWS : (i + 1) * ROWS], "gp": gsw, "ident": idn}
        for i in range(NCORES)
    ]
    res = run_bass_kernel_spmd(
        nc, in_maps, core_ids=list(range(NCORES)), trace=TRACE
    )
    LAST_RESULT = res
    return np.concatenate([r["out"] for r in res.results], axis=0)


# revision 19
# speedup vs baseline: 1.0312x; 1.0312x over previous
"""BCH/RS systematic encoder kernel for Trainium2 (8 NeuronCores, data parallel).

Computes out = concat([msg, (msg @ Gp) mod 2], axis=-1) for
msg [16384, 1000] f32 of 0/1 bits and Gp [1000, 256] f32 of 0/1 bits.

v6 design (per core, 2048 rows, 8 superchunks of 2x128 rows). Earlier
xbar-transpose variants all serialized: the Tile scheduler cross-
serializes HWDGE DMAs against in-flight DMA_TRANSPOSEs (deadlock guard),
and the xbar's thousands of tiny packets starve any concurrent bulk
stream at the SDMA packet round-robin. So: NO DMA transposes at all.

  - HWDGE f32 loads (sync ring, all emitted upfront, in-order drain)
  - HWDGE f32 msg-stores (scalar ring) a -> out[:, :1000]; pure stream,
    never waits on compute
  - DVE cast a -> abf bf16 (0/1 exact), pad cols 1000:1024 memset
  - PE transposes (matmul transpose-mode vs identity): 16 tiles
    [128m,128k] -> PSUM [128k,128m] per superchunk, in 4 groups of 4
  - ACT evicts each PSUM group -> b [k, m] bf16 SBUF (also ACT's only job)
  - 16 accumulating bf16 matmuls: acc[m, 2*256] += b_k.T @ Gp_k
  - PE stream interleaves T(it+1) with mm(it) so the Tensor engine never
    idles: TRN2 PE p-states are 0.65/1.2/2.4 GHz and only reach 2.4 GHz
    after ~3us of CONTINUOUS execution (gapped bursts run at 1.2 GHz)
  - parity mod-2 on DVE: CAST psum f32->i32, AND 1, CAST i32->f32
  - HWDGE parity stores (sync ring, after all loads)

HBM traffic/core = 8.19 MB read + 10.29 MB write (the minimum); measured
single-direction stream rate ~400 GB/s/core -> ~46 us stream floor.
"""

import os
import sys

import numpy as np

if os.path.isdir("/opt/trn_rl_repo") and "/opt/trn_rl_repo" not in sys.path:
    sys.path.insert(0, "/opt/trn_rl_repo")

import ml_dtypes

import concourse.bacc as bacc
import concourse.mybir as mybir
import concourse.tile as tile
from concourse.bass_utils import run_bass_kernel_spmd

BATCH = 16384
MSG = 1000
NPAR = 256
NCORES = 8
ROWS = BATCH // NCORES  # 2048
P = 128
KCH = 8  # k chunks; padded K = 1024
KPAD = KCH * P
SC = 2  # m-chunks per superchunk
NG = SC * KCH // 4  # transpose groups of 4 tiles per superchunk

# test.py pokes these for profiling
TRACE = False
LAST_RESULT = None

_CACHE = {}


def build_nc(rows=ROWS):
    """Emit the Bass/Tile IR for one core handling `rows` rows."""
    n_it = rows // (SC * P)
    nc = bacc.Bacc("TRN2", target_bir_lowering=False, debug=False)
    msg = nc.dram_tensor("msg", [rows, MSG], mybir.dt.float32, kind="ExternalInput")
    gp = nc.dram_tensor("gp", [P, KCH * NPAR], mybir.dt.bfloat16, kind="ExternalInput")
    ident = nc.dram_tensor("ident", [P, P], mybir.dt.float32, kind="ExternalInput")
    out = nc.dram_tensor(
        "out", [rows, MSG + NPAR], mybir.dt.float32, kind="ExternalOutput"
    )

    msg3 = msg[:, :].rearrange("(s c p) k -> s c p k", c=SC, p=P)
    out3 = out[:, :].rearrange("(s c p) k -> s c p k", c=SC, p=P)

    with tile.TileContext(nc) as tc:
        with (
            # every buffer's full live range fits in its pool, so the Tile
            # scheduler never emits slot-release relay waits (those ride
            # engine queues and serialize the pipeline cross-superchunk)
            tc.tile_pool(name="gpool", bufs=1) as gpool,
            tc.tile_pool(name="apool", bufs=n_it + 1) as apool,
            tc.tile_pool(name="bpool", bufs=6) as bpool,
            tc.tile_pool(name="parpool", bufs=n_it) as parpool,
            tc.tile_pool(name="ipool", bufs=4) as ipool,
            tc.tile_pool(name="tpool", bufs=6, space="PSUM") as tpool,
            tc.tile_pool(name="ppool", bufs=2, space="PSUM") as ppool,
        ):
            # constants on the SWDGE queue: the HWDGE lane pool (8 sems,
            # round-robin, re-armed on reuse) then carries ONLY 8 loads +
            # 8 msg-stores -> loads take the first 8 lanes and never wait,
            # stores arm on long-completed loads
            idsb = gpool.tile([P, P], mybir.dt.float32)
            nc.gpsimd.dma_start(out=idsb[:, :], in_=ident[:, :])
            gsb = gpool.tile([P, KCH * NPAR], mybir.dt.bfloat16)
            nc.gpsimd.dma_start(out=gsb[:, :], in_=gp[:, :])

            a_t = {}
            b_t = {}
            acc_t = {}

            def emit_load(it):
                # f32 tile with pad cols 1000:1024 (memset once; the PE
                # transposes read the pad, matmuls contract only k<1000)
                a = apool.tile([P, SC, KPAD], mybir.dt.float32, tag="a")
                nc.sync.dma_start(
                    out=a[:, :, 0:MSG],
                    in_=msg3[it, :, :, :].rearrange("c p k -> p c k"),
                )
                a_t[it] = a

            def emit_msg_store(it):
                # model-time offset: keeps the scheduler's global tick order
                # (which drives HWDGE sem-lane assignment) from interleaving
                # store lanes among the load lanes -> loads never arm-wait
                # on store completions
                with tc.tile_wait_until(0.030 + 0.002 * it):
                    nc.scalar.dma_start(
                        out=out3[it, :, :, 0:MSG].rearrange("c p k -> p c k"),
                        in_=a_t[it][:, :, 0:MSG],
                    )


            def emit_trans(it):
                # PE transpose-mode straight from the f32 a-tile in 4-tile
                # groups -> PSUM f32; evictions (f32 -> bf16 lhsT layout
                # b[q, c*KCH+kb, m]) split across ACT and DVE so neither
                # becomes the serial hot engine
                a = a_t[it]
                b = bpool.tile([P, SC * KCH, P], mybir.dt.bfloat16, tag="b")
                for g in range(NG):
                    t = tpool.tile([P, 4, P], mybir.dt.float32, tag="t")
                    for j in range(4):
                        c, kb = divmod(g * 4 + j, KCH)
                        nc.tensor.transpose(
                            t[:, j, :],
                            a[:, c, kb * P : (kb + 1) * P],
                            idsb[:, :],
                        )
                    if g % 2 == 0:
                        nc.scalar.copy(b[:, g * 4 : (g + 1) * 4, :], t[:, :, :])
                    else:
                        nc.vector.tensor_copy(
                            b[:, g * 4 : (g + 1) * 4, :], t[:, :, :]
                        )
                b_t[it] = b

            def emit_mm(it):
                b = b_t.pop(it)
                acc = ppool.tile([P, SC * NPAR], mybir.dt.float32, tag="acc")
                for c in range(SC):
                    for kb in range(KCH):
                        kk = P if kb < KCH - 1 else MSG - (KCH - 1) * P  # 104 tail
                        nc.tensor.matmul(
                            acc[:, c * NPAR : (c + 1) * NPAR],
                            b[0:kk, c * KCH + kb, :],
                            gsb[0:kk, kb * NPAR : (kb + 1) * NPAR],
                            start=(kb == 0),
                            stop=(kb == KCH - 1),
                        )
                acc_t[it] = acc

            def emit_parity(it):
                # mod 2 on DVE only: psum f32 -> i32 (numeric cast, exact
                # for integer sums <= 1000), AND 1, i32 -> f32
                ci = ipool.tile([P, SC * NPAR], mybir.dt.int32, tag="ci")
                nc.scalar.copy(ci[:, :], acc_t.pop(it)[:, :])
                nc.vector.tensor_scalar(
                    ci[:, :], ci[:, :], 1, None, mybir.AluOpType.bitwise_and
                )
                par = parpool.tile([P, SC, NPAR], mybir.dt.float32, tag="par")
                nc.vector.tensor_copy(
                    par[:, :, :].rearrange("p c n -> p (c n)"), ci[:, :]
                )
                return par

            def emit_par_store(it, par):
                # SWDGE: fires right after parity lands instead of queueing
                # behind the load/store streams on the HWDGE rings
                nc.gpsimd.dma_start(
                    out=out3[it, :, :, MSG : MSG + NPAR].rearrange("c p k -> p c k"),
                    in_=par[:, :, :],
                )

            for i in range(n_it):
                emit_load(i)
            for i in range(n_it):
                nc.vector.memset(a_t[i][:, :, MSG:KPAD], 0)
            emit_trans(0)
            for it in range(n_it):
                emit_msg_store(it)
                if it + 1 < n_it:
                    emit_trans(it + 1)
                emit_mm(it)
                par = emit_parity(it)
                emit_par_store(it, par)

    nc.compile()
    return nc


def prep_gp(Gp):
    """Pad Gp to 1024 rows and swizzle to the [128, 8*256] bf16 SBUF layout."""
    gp = np.asarray(Gp, dtype=np.float32)
    gp_pad = np.zeros((KPAD, NPAR), dtype=np.float32)
    gp_pad[:MSG] = gp
    gsw = gp_pad.reshape(KCH, P, NPAR).transpose(1, 0, 2).reshape(P, KCH * NPAR)
    return np.ascontiguousarray(gsw).astype(ml_dtypes.bfloat16)


def prep_ident():
    return np.eye(P, dtype=np.float32)


def kernel(message_bits, Gp):
    global LAST_RESULT
    msg = np.ascontiguousarray(np.asarray(message_bits, dtype=np.float32))
    assert msg.shape == (BATCH, MSG), msg.shape
    gsw = prep_gp(Gp)
    idn = prep_ident()

    if "nc" not in _CACHE:
        _CACHE["nc"] = build_nc()
    nc = _CACHE["nc"]

    in_maps = [
        {"msg": msg[i * ROWS : (i + 1) * ROWS], "gp": gsw, "ident": idn}
        for i in range(NCORES)
    ]
    res = run_bass_kernel_spmd(
        nc, in_maps, core_ids=list(range(NCORES)), trace=TRACE
    )
    LAST_RESULT = res
    return np.concatenate([r["out"] for r in res.results], axis=0)


# revision 20
# speedup vs baseline: 1.1204x; 1.0865x over previous
"""BCH/RS systematic encoder kernel for Trainium2 (8 NeuronCores, data parallel).

Computes out = concat([msg, (msg @ Gp) mod 2], axis=-1) for
msg [16384, 1000] f32 of 0/1 bits and Gp [1000, 256] f32 of 0/1 bits.

v6 design (per core, 2048 rows, 8 superchunks of 2x128 rows). Earlier
xbar-transpose variants all serialized: the Tile scheduler cross-
serializes HWDGE DMAs against in-flight DMA_TRANSPOSEs (deadlock guard),
and the xbar's thousands of tiny packets starve any concurrent bulk
stream at the SDMA packet round-robin. So: NO DMA transposes at all.

  - HWDGE f32 loads (sync ring, all emitted upfront, in-order drain)
  - HWDGE f32 msg-stores (scalar ring) a -> out[:, :1000]; pure stream,
    never waits on compute
  - DVE cast a -> abf bf16 (0/1 exact), pad cols 1000:1024 memset
  - PE transposes (matmul transpose-mode vs identity): 16 tiles
    [128m,128k] -> PSUM [128k,128m] per superchunk, in 4 groups of 4
  - ACT evicts each PSUM group -> b [k, m] bf16 SBUF (also ACT's only job)
  - 16 accumulating bf16 matmuls: acc[m, 2*256] += b_k.T @ Gp_k
  - PE stream interleaves T(it+1) with mm(it) so the Tensor engine never
    idles: TRN2 PE p-states are 0.65/1.2/2.4 GHz and only reach 2.4 GHz
    after ~3us of CONTINUOUS execution (gapped bursts run at 1.2 GHz)
  - parity mod-2 on DVE: CAST psum f32->i32, AND 1, CAST i32->f32
  - HWDGE parity stores (sync ring, after all loads)

HBM traffic/core = 8.19 MB read + 10.29 MB write (the minimum); measured
single-direction stream rate ~400 GB/s/core -> ~46 us stream floor.
"""

import os
import sys

import numpy as np

if os.path.isdir("/opt/trn_rl_repo") and "/opt/trn_rl_repo" not in sys.path:
    sys.path.insert(0, "/opt/trn_rl_repo")

import ml_dtypes

import concourse.bacc as bacc
import concourse.mybir as mybir
import concourse.tile as tile
from concourse.bass_utils import run_bass_kernel_spmd

BATCH = 16384
MSG = 1000
NPAR = 256
NCORES = 8
ROWS = BATCH // NCORES  # 2048
P = 128
KCH = 8  # k chunks; padded K = 1024
KPAD = KCH * P
SC = 2  # m-chunks per superchunk
NG = SC * KCH // 4  # transpose groups of 4 tiles per superchunk

# test.py pokes these for profiling
TRACE = False
LAST_RESULT = None

_CACHE = {}


def build_nc(rows=ROWS):
    """Emit the Bass/Tile IR for one core handling `rows` rows."""
    n_it = rows // (SC * P)
    nc = bacc.Bacc("TRN2", target_bir_lowering=False, debug=False)
    msg = nc.dram_tensor("msg", [rows, MSG], mybir.dt.float32, kind="ExternalInput")
    gp = nc.dram_tensor("gp", [P, KCH * NPAR], mybir.dt.bfloat16, kind="ExternalInput")
    ident = nc.dram_tensor("ident", [P, P], mybir.dt.float32, kind="ExternalInput")
    out = nc.dram_tensor(
        "out", [rows, MSG + NPAR], mybir.dt.float32, kind="ExternalOutput"
    )

    msg3 = msg[:, :].rearrange("(s c p) k -> s c p k", c=SC, p=P)
    out3 = out[:, :].rearrange("(s c p) k -> s c p k", c=SC, p=P)

    with tile.TileContext(nc) as tc:
        with (
            # every buffer's full live range fits in its pool, so the Tile
            # scheduler never emits slot-release relay waits (those ride
            # engine queues and serialize the pipeline cross-superchunk)
            tc.tile_pool(name="gpool", bufs=1) as gpool,
            tc.tile_pool(name="apool", bufs=n_it + 1) as apool,
            tc.tile_pool(name="bpool", bufs=6) as bpool,
            tc.tile_pool(name="parpool", bufs=n_it) as parpool,
            tc.tile_pool(name="ipool", bufs=4) as ipool,
            tc.tile_pool(name="tpool", bufs=4, space="PSUM") as tpool,
            tc.tile_pool(name="ppool", bufs=4, space="PSUM") as ppool,
        ):
            # constants on the SWDGE queue: the HWDGE lane pool (8 sems,
            # round-robin, re-armed on reuse) then carries ONLY 8 loads +
            # 8 msg-stores -> loads take the first 8 lanes and never wait,
            # stores arm on long-completed loads
            idsb = gpool.tile([P, P], mybir.dt.float32)
            nc.gpsimd.dma_start(out=idsb[:, :], in_=ident[:, :])
            gsb = gpool.tile([P, KCH * NPAR], mybir.dt.bfloat16)
            nc.gpsimd.dma_start(out=gsb[:, :], in_=gp[:, :])

            a_t = {}
            b_t = {}
            acc_t = {}

            def emit_load(it):
                # f32 tile with pad cols 1000:1024 (memset once; the PE
                # transposes read the pad, matmuls contract only k<1000)
                a = apool.tile([P, SC, KPAD], mybir.dt.float32, tag="a")
                nc.sync.dma_start(
                    out=a[:, :, 0:MSG],
                    in_=msg3[it, :, :, :].rearrange("c p k -> p c k"),
                )
                a_t[it] = a

            def emit_msg_store(it):
                # model-time offset: keeps the scheduler's global tick order
                # (which drives HWDGE sem-lane assignment) from interleaving
                # store lanes among the load lanes -> loads never arm-wait
                # on store completions
                with tc.tile_wait_until(0.030 + 0.002 * it):
                    nc.scalar.dma_start(
                        out=out3[it, :, :, 0:MSG].rearrange("c p k -> p c k"),
                        in_=a_t[it][:, :, 0:MSG],
                    )


            def emit_trans(it):
                # PE transpose-mode straight from the f32 a-tile in 4-tile
                # groups -> PSUM f32; evictions (f32 -> bf16 lhsT layout
                # b[q, c*KCH+kb, m]) split across ACT and DVE so neither
                # becomes the serial hot engine
                a = a_t[it]
                b = bpool.tile([P, SC * KCH, P], mybir.dt.bfloat16, tag="b")
                for g in range(NG):
                    t = tpool.tile([P, 4, P], mybir.dt.float32, tag="t")
                    for j in range(4):
                        c, kb = divmod(g * 4 + j, KCH)
                        nc.tensor.transpose(
                            t[:, j, :],
                            a[:, c, kb * P : (kb + 1) * P],
                            idsb[:, :],
                        )
                    if g % 2 == 0:
                        nc.scalar.copy(b[:, g * 4 : (g + 1) * 4, :], t[:, :, :])
                    else:
                        nc.vector.tensor_copy(
                            b[:, g * 4 : (g + 1) * 4, :], t[:, :, :]
                        )
                b_t[it] = b

            def emit_mm(it):
                b = b_t.pop(it)
                acc = ppool.tile([P, SC * NPAR], mybir.dt.float32, tag="acc")
                for c in range(SC):
                    for kb in range(KCH):
                        kk = P if kb < KCH - 1 else MSG - (KCH - 1) * P  # 104 tail
                        nc.tensor.matmul(
                            acc[:, c * NPAR : (c + 1) * NPAR],
                            b[0:kk, c * KCH + kb, :],
                            gsb[0:kk, kb * NPAR : (kb + 1) * NPAR],
                            start=(kb == 0),
                            stop=(kb == KCH - 1),
                        )
                acc_t[it] = acc

            def emit_parity(it):
                # mod 2 on DVE only: psum f32 -> i32 (numeric cast, exact
                # for integer sums <= 1000), AND 1, i32 -> f32
                ci = ipool.tile([P, SC * NPAR], mybir.dt.int32, tag="ci")
                nc.scalar.copy(ci[:, :], acc_t.pop(it)[:, :])
                nc.vector.tensor_scalar(
                    ci[:, :], ci[:, :], 1, None, mybir.AluOpType.bitwise_and
                )
                par = parpool.tile([P, SC, NPAR], mybir.dt.float32, tag="par")
                nc.vector.tensor_copy(
                    par[:, :, :].rearrange("p c n -> p (c n)"), ci[:, :]
                )
                return par

            def emit_par_store(it, par):
                # SWDGE: fires right after parity lands instead of queueing
                # behind the load/store streams on the HWDGE rings
                nc.gpsimd.dma_start(
                    out=out3[it, :, :, MSG : MSG + NPAR].rearrange("c p k -> p c k"),
                    in_=par[:, :, :],
                )

            for i in range(n_it):
                emit_load(i)
            for i in range(n_it):
                nc.vector.memset(a_t[i][:, :, MSG:KPAD], 0)
            emit_trans(0)
            for it in range(n_it):
                emit_msg_store(it)
                if it + 1 < n_it:
                    emit_trans(it + 1)
                emit_mm(it)
                par = emit_parity(it)
                emit_par_store(it, par)

    nc.compile()
    return nc


def prep_gp(Gp):
    """Pad Gp to 1024 rows and swizzle to the [128, 8*256] bf16 SBUF layout."""
    gp = np.asarray(Gp, dtype=np.float32)
    gp_pad = np.zeros((KPAD, NPAR), dtype=np.float32)
    gp_pad[:MSG] = gp
    gsw = gp_pad.reshape(KCH, P, NPAR).transpose(1, 0, 2).reshape(P, KCH * NPAR)
    return np.ascontiguousarray(gsw).astype(ml_dtypes.bfloat16)


def prep_ident():
    return np.eye(P, dtype=np.float32)


def kernel(message_bits, Gp):
    global LAST_RESULT
    msg = np.ascontiguousarray(np.asarray(message_bits, dtype=np.float32))
    assert msg.shape == (BATCH, MSG), msg.shape
    gsw = prep_gp(Gp)
    idn = prep_ident()

    if "nc" not in _CACHE:
        _CACHE["nc"] = build_nc()
    nc = _CACHE["nc"]

    in_maps = [
        {"msg": msg[i * ROWS : (i + 1) * ROWS], "gp": gsw, "ident": idn}
        for i in range(NCORES)
    ]
    res = run_bass_kernel_spmd(
        nc, in_maps, core_ids=list(range(NCORES)), trace=TRACE
    )
    LAST_RESULT = res
    return np.concatenate([r["out"] for r in res.results], axis=0)
